# revision 7
# baseline (speedup 1.0000x reference)
"""Causal multi-head attention (RMSNorm + QKV + causal softmax + out-proj)
for Trainium2, sharded over 8 NeuronCores. v1.5.

Sharding: data-parallel over batch (2) x tensor-parallel over head-groups
(16 heads -> 4 groups of 4). Core c = 4*b + hg computes
    partial_out[b] = Attn_heads[4hg:4hg+4](x[b]) @ Wo[256hg:256hg+256, :]
and the host sums the 4 head-group partials per batch (bf16 partials,
f32 accumulation on host).

Front end (v2): host supplies x^T and x in bf16 -> no PE transposes of
activations; RMSNorm scales are folded (s = 32/||x_row|| applied to
Q^T/K^T columns via a PE broadcast of s, and to V rows per-partition,
also folding the key mask); gamma folded into weights on host;
s = exp(-0.5*ln(ss/1024)) so every Act op lives in the natural_log_exp
table set (no table swaps; Square rides along as a filler function).

Attention core (v1, proven): per (qc, hp, kb): S^T = K^T.T Q^T (two
K=64 head matmuls row-packed via tile_position), P^T = exp(S^T/8) (no
bias: mask folded into V and the ones column), causal via block
skipping + triangular multiply on diagonal windows, O^T accumulated as
Vaug^T P^T with a ones column giving row sums z; normalize with
reciprocal straight off the PSUM z-row + PE broadcast + DVE multiply.
Out-proj over A^T halves, bf16 partials DMA'd out.
"""
import os
import sys
import contextlib

for _p in ("/opt/trn_rl_repo", os.path.expanduser("~/.axon_site/_ro/trn_rl_repo")):
    if os.path.isdir(_p) and _p not in sys.path:
        sys.path.insert(0, _p)

import numpy as np

B = 2
N = 2048
DIM = 1024
HEADS = 16
DH = 64
SCALE = DH ** -0.5   # 0.125
NCORES = 8
NGROUPS = 4          # head groups (tensor parallel)
HPC = HEADS // NGROUPS  # 4 heads per core
P = 128
RC = 4               # q-chunks of 512
QCHUNK = 512
NKB = N // P         # 16 key blocks
REPS = 1             # timing aid: emit the compute body REPS times
LOOP_N = 0           # timing aid: if >0, wrap the body in a hardware loop


def _build():
    import concourse.bass as bass
    import concourse.mybir as mybir
    import concourse.tile as tile
    from concourse import bacc

    dt = mybir.dt
    f32 = dt.float32
    f32r = dt.float32r
    bf16 = dt.bfloat16
    AF = mybir.ActivationFunctionType
    ALU = mybir.AluOpType

    nc = bacc.Bacc("TRN2", target_bir_lowering=False, debug=False,
                   num_devices=NCORES)

    xt_d = nc.dram_tensor("xt", [DIM, N], bf16, kind="ExternalInput")
    xn_d = nc.dram_tensor("xn", [N, DIM], bf16, kind="ExternalInput")
    wq_d = nc.dram_tensor("wq", [DIM, HPC * DH], bf16, kind="ExternalInput")
    wk_d = nc.dram_tensor("wk", [DIM, HPC * DH], bf16, kind="ExternalInput")
    wv_d = nc.dram_tensor("wv", [DIM, HPC * DH], bf16, kind="ExternalInput")
    wo_d = nc.dram_tensor("wo", [HPC * DH, DIM], bf16, kind="ExternalInput")
    tri_d = nc.dram_tensor("tri", [P, P], bf16, kind="ExternalInput")
    id_d = nc.dram_tensor("ident", [P, P], f32, kind="ExternalInput")
    on_d = nc.dram_tensor("onesin", [1, P], f32, kind="ExternalInput")
    km_d = nc.dram_tensor("kmask", [P, NKB], f32, kind="ExternalInput")
    vo_d = nc.dram_tensor("vones", [P, NKB * HPC], bf16, kind="ExternalInput")
    out_d = nc.dram_tensor("out", [N, DIM], bf16, kind="ExternalOutput")

    with tile.TileContext(nc) as tc:
        with (
            tc.tile_pool(name="consts", bufs=1) as consts,
            tc.tile_pool(name="wpool", bufs=1) as wpool,
            tc.tile_pool(name="big", bufs=1) as big,
        ):
            # ---- constants / weights (preamble, outside the timed body)
            tri = consts.tile([P, P], bf16)
            nc.gpsimd.dma_start(tri[:], tri_d[:])
            ident = consts.tile([P, P], f32)
            nc.gpsimd.dma_start(ident[:], id_d[:])
            ones1 = consts.tile([1, P], f32)
            nc.gpsimd.dma_start(ones1[:], on_d[:])
            onesr = consts.tile([1, DH], f32r)
            nc.gpsimd.dma_start(onesr[:], on_d[0:1, 0:DH])
            kmask = consts.tile([P, NKB], f32)
            nc.sync.dma_start(kmask[:], km_d[:])

            wq = wpool.tile([P, 8, HPC * DH], bf16)
            wk = wpool.tile([P, 8, HPC * DH], bf16)
            wv = wpool.tile([P, 8, HPC * DH], bf16)
            nc.gpsimd.dma_start(wq[:], wq_d.ap().rearrange("(k p) c -> p k c", p=P))
            nc.gpsimd.dma_start(wk[:], wk_d.ap().rearrange("(k p) c -> p k c", p=P))
            nc.gpsimd.dma_start(wv[:], wv_d.ap().rearrange("(k p) c -> p k c", p=P))
            wo = wpool.tile([P, 2, DIM], bf16)
            nc.gpsimd.dma_start(wo[:], wo_d.ap().rearrange("(hp p) c -> p hp c", p=P))

            # ---- persistent activations
            xt = big.tile([P, 8, N], bf16)
            qt = big.tile([P, 2, N], bf16)
            kt = big.tile([P, 2, N], bf16)
            vt = big.tile([P, NKB, HPC, DH + 1], bf16)
            a0 = big.tile([P, N], bf16)          # A^T head pair 0
            a1 = big.tile([P, N], bf16)
            sbc = big.tile([P, N], f32)
            ss = big.tile([P, NKB], f32)
            lnss = big.tile([P, NKB], f32)
            sfacT = big.tile([P, NKB], f32)
            smaskT = big.tile([P, NKB], f32)
            srow = big.tile([1, NKB, P], f32)

            nc.gpsimd.dma_start(
                vt[:, :, :, DH:DH + 1],
                vo_d.ap().rearrange("p (kb h) -> p kb h", h=HPC).unsqueeze(3))

            loop_ctx = (
                tc.For_i(0, LOOP_N, 1,
                         hint_engines=(mybir.EngineType.PE,
                                       mybir.EngineType.Activation,
                                       mybir.EngineType.DVE,
                                       mybir.EngineType.Pool,
                                       mybir.EngineType.SP))
                if LOOP_N > 0 else contextlib.nullcontext()
            )
            _ls = contextlib.ExitStack()
            _ls.enter_context(loop_ctx)
            for _rep in range(REPS):
                with (
                    tc.tile_pool(name="xin", bufs=4) as xin,
                    tc.tile_pool(name="sq", bufs=2) as sqp,
                    tc.tile_pool(name="pt", bufs=5) as ptp,
                    tc.tile_pool(name="nrm", bufs=2) as nrm,
                    tc.tile_pool(name="outp", bufs=2) as outp,
                    tc.tile_pool(name="ps1", bufs=2, space="PSUM") as ps1,
                    tc.tile_pool(name="sps", bufs=2, space="PSUM") as sps,
                    tc.tile_pool(name="ops", bufs=1, space="PSUM") as ops,
                ):
                    # ---- x^T load (for projections)
                    for k in range(8):
                        nc.gpsimd.dma_start(xt[:, k, :], xt_d[k * P:(k + 1) * P, :])
                    # ---- merged per-chunk pipeline: stats -> QKV -> attention
                    for qc in range(RC):
                        qs = slice(qc * QCHUNK, (qc + 1) * QCHUNK)
                        nkb = 4 * qc + 4
                        tsl = slice(qc * 4, qc * 4 + 4)
                        # row stats for this chunk's 4 seq tiles
                        for t in range(qc * 4, qc * 4 + 4):
                            xnt_ = xin.tile([P, DIM], bf16, tag="xn")
                            nc.sync.dma_start(xnt_[:], xn_d[t * P:(t + 1) * P, :])
                            sq = sqp.tile([P, DIM], bf16, tag="sq")
                            nc.scalar.activation(sq[:], xnt_[:], AF.Square,
                                                 accum_out=ss[:, t:t + 1])
                        # s = 32/sqrt(ss) = exp(-0.5*ln(ss/1024)): stays in
                        # the natural_log_exp act table set (no table swaps).
                        nc.scalar.activation(lnss[:, tsl], ss[:, tsl], AF.Ln,
                                             scale=1.0 / DIM)
                        nc.scalar.activation(sfacT[:, tsl], lnss[:, tsl],
                                             AF.Exp, scale=-0.5)
                        nc.vector.tensor_tensor(smaskT[:, tsl], sfacT[:, tsl],
                                                kmask[:, tsl], ALU.mult)
                        # s as partition-0 rows, then broadcast across parts
                        for j in range(4):
                            kb = qc * 4 + j
                            tps = ps1.tile([P, 4, P], f32, tag="ps1")
                            nc.tensor.transpose(tps[0:1, 0, 0:P],
                                                sfacT[:, kb:kb + 1], ident[:])
                            nc.vector.tensor_copy(srow[0:1, kb, :],
                                                  tps[0:1, 0, 0:P])
                        ps = ps1.tile([P, 4, P], f32, tag="ps1")
                        for j in range(4):
                            nc.tensor.matmul(ps[:, j, :], ones1[:],
                                             srow[0:1, qc * 4 + j, :],
                                             start=True, stop=True)
                        nc.vector.tensor_copy(
                            sbc[:, qs], ps[:].rearrange("p a b -> p (a b)"))

                        # Q/K projections for this chunk
                        for w, dst in ((wq, qt), (wk, kt)):
                            for hp in range(2):
                                psq = ps1.tile([P, QCHUNK], f32, tag="ps1")
                                for k in range(8):
                                    nc.tensor.matmul(
                                        psq[:], w[:, k, hp * P:(hp + 1) * P],
                                        xt[:, k, qs],
                                        start=(k == 0), stop=(k == 7))
                                nc.vector.tensor_tensor(
                                    dst[:, hp, qs], psq[:], sbc[:, qs],
                                    ALU.mult)
                        # V projection for this chunk's 4 key tiles
                        for t in range(qc * 4, qc * 4 + 4):
                            psv = ps1.tile([P, HPC * DH], f32, tag="ps1")
                            for k in range(8):
                                nc.tensor.matmul(
                                    psv[:], xt[:, k, t * P:(t + 1) * P],
                                    wv[:, k, :],
                                    start=(k == 0), stop=(k == 7))
                            nc.vector.tensor_scalar(
                                out=vt[:, t, :, 0:DH],
                                in0=psv[:].rearrange("p (h d) -> p h d", d=DH),
                                scalar1=smaskT[:, t:t + 1], scalar2=None,
                                op0=ALU.mult)

                        # attention for q-chunk qc (v1 interleaved core)
                        for hp, adst in ((0, a0), (1, a1)):
                            ot = ops.tile([DH + 1, 2, QCHUNK], f32, tag="o")
                            for kb in range(nkb):
                                ks = slice(kb * P, (kb + 1) * P)
                                o = max(0, kb * P - qc * QCHUNK)
                                qso = slice(qc * QCHUNK + o, (qc + 1) * QCHUNK)
                                st = sps.tile([P, 2, QCHUNK], f32, tag="s")
                                for h in range(2):
                                    nc.tensor.matmul(
                                        st[:, h, o:],
                                        kt[h * DH:(h + 1) * DH, hp, ks],
                                        qt[h * DH:(h + 1) * DH, hp, qso],
                                        start=True, stop=True,
                                        tile_position=(h * DH, 0))
                                pt = ptp.tile([P, 2, QCHUNK], bf16, tag="pt")
                                nc.scalar.activation(pt[:, :, o:], st[:, :, o:],
                                                     AF.Exp, scale=SCALE)
                                if kb >= 4 * qc:  # diagonal: triangular mask
                                    nc.vector.tensor_tensor(
                                        pt[:, :, o:o + P], pt[:, :, o:o + P],
                                        tri[:, None, :].broadcast_to([P, 2, P]),
                                        ALU.mult)
                                for h in range(2):
                                    nc.tensor.matmul(
                                        ot[:, h, o:], vt[:, kb, 2 * hp + h, :],
                                        pt[:, h, o:],
                                        start=(kb == 0), stop=(kb == nkb - 1),
                                        skip_group_check=True)
                            # normalize: A = O[0:64] * (1 / O[64])
                            rec = nrm.tile([1, 2, QCHUNK], f32r, tag="rec")
                            with nc.allow_low_precision(reason="f32r softmax recip"):
                                nc.vector.reciprocal(rec[:], ot[DH:DH + 1, :, :])
                            bt = sps.tile([DH, 2, QCHUNK], f32, tag="s")
                            for h in range(2):
                                nc.tensor.matmul(bt[:, h, :], onesr[:],
                                                 rec[0:1, h, :],
                                                 start=True, stop=True)
                            osb = nrm.tile([DH, 2, QCHUNK], f32, tag="osb")
                            nc.vector.tensor_copy(osb[:], ot[0:DH, :, :])
                            nc.vector.tensor_tensor(adst[0:DH, qs], osb[:, 0, :],
                                                    bt[:, 0, :], ALU.mult)
                            ashq = nrm.tile([DH, QCHUNK], bf16, tag="ashq")
                            nc.vector.tensor_tensor(ashq[:], osb[:, 1, :],
                                                    bt[:, 1, :], ALU.mult)
                            nc.sync.dma_start(adst[DH:2 * DH, qs], ashq[:])

                        # out-proj rows of this q-chunk: overlaps the next
                        # chunk's attention; DMAs alternate queues
                        for r in range(qc * 4, qc * 4 + 4):
                            rs = slice(r * P, (r + 1) * P)
                            orow = outp.tile([P, DIM], bf16, tag="orow")
                            for cc in range(2):
                                cs = slice(cc * QCHUNK, (cc + 1) * QCHUNK)
                                ps = ps1.tile([P, QCHUNK], f32, tag="ps1")
                                for hp2, a in ((0, a0), (1, a1)):
                                    nc.tensor.matmul(
                                        ps[:], a[:, rs], wo[:, hp2, cs],
                                        start=(hp2 == 0), stop=(hp2 == 1))
                                nc.vector.tensor_copy(orow[:, cs], ps[:])
                            eng = nc.sync if r % 2 == 0 else nc.gpsimd
                            eng.dma_start(out_d[rs, :], orow[:])
            _ls.close()

    nc.compile()
    return nc


_CACHE = {}


def _get_nc():
    if "nc" not in _CACHE:
        _CACHE["nc"] = _build()
    return _CACHE["nc"]


def _bf16(a):
    import ml_dtypes
    return np.ascontiguousarray(np.asarray(a, dtype=ml_dtypes.bfloat16))


def kernel(x, mask, gamma, Wq, Wkv, Wo):
    from concourse import bass_utils

    x = np.asarray(x, dtype=np.float32)
    mask = np.asarray(mask)
    gamma = np.asarray(gamma, dtype=np.float32)
    Wq = np.asarray(Wq, dtype=np.float32) * gamma[:, None]
    Wk = np.asarray(Wkv[:, :HEADS * DH], dtype=np.float32) * gamma[:, None]
    Wv = np.asarray(Wkv[:, HEADS * DH:], dtype=np.float32) * gamma[:, None]
    Wo = np.asarray(Wo, dtype=np.float32)

    tri = (np.arange(P)[None, :] >= np.arange(P)[:, None]).astype(np.float32)
    ident = np.eye(P, dtype=np.float32)

    in_maps = []
    for c in range(NCORES):
        b, hg = divmod(c, NGROUPS)
        cs = slice(hg * HPC * DH, (hg + 1) * HPC * DH)
        m = mask[b].astype(np.float32)
        mT = np.ascontiguousarray(m.reshape(NKB, P).T)
        in_maps.append({
            "xt": _bf16(x[b].T),
            "xn": _bf16(x[b]),
            "wq": _bf16(Wq[:, cs]),
            "wk": _bf16(Wk[:, cs]),
            "wv": _bf16(Wv[:, cs]),
            "wo": _bf16(Wo[cs, :]),
            "tri": _bf16(tri),
            "ident": ident,
            "onesin": np.ones((1, P), dtype=np.float32),
            "kmask": np.ascontiguousarray(mT),
            "vones": _bf16(np.ascontiguousarray(
                np.broadcast_to(mT[:, :, None],
                                (P, NKB, HPC)).reshape(P, NKB * HPC))),
        })

    nc = _get_nc()
    _CACHE["last_in_maps"] = in_maps
    res = bass_utils.run_bass_kernel_spmd(nc, in_maps, core_ids=list(range(NCORES)))
    out = np.zeros((B, N, DIM), dtype=np.float32)
    for c in range(NCORES):
        b = c // NGROUPS
        out[b] += np.asarray(res.results[c]["out"]).astype(np.float32)
    return out
